# revision 15
# baseline (speedup 1.0000x reference)
"""Trainium2 Bass kernel for nn_DWTExtractor.

Computes, for each single-channel 1024x1024 image, 6 output channels
(3 Haar DWT2 details + 3 Coif1 DWT2 details bilinearly resized to 512x512).

Everything is linear and separable, so each channel is
    chan = RowM @ img @ ColM^T
with RowM/ColM in {Hlo, Hhi, RClo, RChi} (all [512, 1024] banded operators;
RC* fold the coif1 DWT with the jax.image.resize 514->512 linear+antialias
matrix). Both passes run on the TensorEngine with the *data* as the
stationary operand (lhsT), so each pass's output lands in PSUM already
transposed for the next pass - no transpose instructions at all:

  pass 1: T2[op][w, n] = sum_h X[h, w] * Op[n, h]
          lhsT = X[h-window, w-window] (128x128), rhs = packed band matrix
  pass 2: chan[m, n]   = sum_w T2[op][w, m] * Col[n, w]
          lhsT = T2[w-window, m-slice], rhs = band slice

The image axes are covered by 9 overlapping 128-wide windows (stride 114)
so that every output column's 12-tap support lies inside a single window;
each window writes a disjoint column slice (singleton PSUM groups, no
cross-window accumulation).

DMA layouts are chosen for few, large, per-partition-contiguous transfers
(the v1 kernel was DMA-packet-count and dma_start-issue bound):
  x dram: [bpc, 128, 9*1024]  - host pre-builds the 9 overlapping h-windows
          (partition p, window t at cols 1024t) -> 1 load per image, 18KB
          contiguous per partition.
  y dram: [bpc, 128, 6*4*512] - partition-major: row (128*rb + p) of channel
          c lives at [p, (4c+rb)*512] -> 2 stores per image (12KB/partition),
          host inverse-permutes.
PSUM->SBUF copies are load-balanced across DVE/Activation/GpSimd, and
pass-2 of image i is interleaved into pass-1 of image i+1 so the in-order
PE queue never drains (PE p-state stays at full clock).

Sharding: pure data parallel, 32 images -> 8 cores x 4 images.
"""

import sys

sys.path.insert(0, "/opt/trn_rl_repo")

from contextlib import ExitStack

import numpy as np

import concourse.bass as bass
from concourse import bacc
import concourse.mybir as mybir
import concourse.tile as tile
from concourse.bass_utils import run_bass_kernel_spmd

# ---------------------------------------------------------------------------
# Host-side operator construction (pure numpy, float64)
# ---------------------------------------------------------------------------

_c = np.array([-0.01565572813546454, -0.0727326195128539, 0.38486484686420286,
               0.8525720202122554, 0.3378976624578092, -0.0727326195128539])
HAAR_LO = np.array([0.7071067811865476, 0.7071067811865476])
HAAR_HI = np.array([-0.7071067811865476, 0.7071067811865476])
COIF1_LO = _c.copy()
COIF1_HI = ((-1.0) ** (np.arange(6) + 1)) * _c[::-1]

H = 1024
NT = 9            # overlapping 128-row windows, stride 114
SLOT = 57         # output columns assigned per window (57 * 9 = 513)
ROW_START = [min(max(114 * t - 6, 0), H - 128) for t in range(NT)]
N_CORES = 8
B_TOTAL = 32
BPC = B_TOTAL // N_CORES

# channel -> (row op index, col op index); ops are [Hlo, Hhi, RClo, RChi]
CHAN_OPS = [(1, 0), (0, 1), (1, 1), (3, 2), (2, 3), (3, 3)]

DT = mybir.dt.float16
NPDT = np.float16
F32 = mybir.dt.float32


def _dwt1d_np(x, filt):
    L = len(filt)
    n = x.shape[-1]
    xp = np.pad(x, [(0, 0)] * (x.ndim - 1) + [(L - 1, L - 1)], mode="symmetric")
    out_len = (n + L - 1) // 2
    fr = filt[::-1]
    y = np.zeros(x.shape[:-1] + (out_len,), dtype=x.dtype)
    for j in range(L):
        y = y + fr[j] * xp[..., 1 + j:1 + j + 2 * out_len:2]
    return y


def _dwt_matrix(n, filt):
    eye = np.eye(n, dtype=np.float64)
    return _dwt1d_np(eye, np.asarray(filt, np.float64)).T.copy()


def _resize_matrix(in_size, out_size):
    """Replicates jax.image.resize(method='linear', antialias=True)."""
    scale = out_size / in_size
    inv_scale = 1.0 / scale
    kernel_scale = max(inv_scale, 1.0)
    sample_f = (np.arange(out_size, dtype=np.float64) + 0.5) * inv_scale - 0.5
    x = np.abs(sample_f[None, :]
               - np.arange(in_size, dtype=np.float64)[:, None]) / kernel_scale
    w = np.maximum(0.0, 1.0 - x)
    total = w.sum(axis=0, keepdims=True)
    w = np.where(np.abs(total) > 1000.0 * np.finfo(np.float32).eps,
                 w / np.where(total != 0, total, 1), 0.0)
    w = np.where(((sample_f >= -0.5) & (sample_f <= in_size - 0.5))[None, :],
                 w, 0.0)
    return w.T.copy()


def build_ops():
    Hlo = _dwt_matrix(H, HAAR_LO)
    Hhi = _dwt_matrix(H, HAAR_HI)
    Clo = _dwt_matrix(H, COIF1_LO)
    Chi = _dwt_matrix(H, COIF1_HI)
    R = _resize_matrix(514, 512)
    return [Hlo, Hhi, R @ Clo, R @ Chi]


def assigned(t):
    return SLOT * t, min(SLOT * (t + 1), 512)


def build_bands(ops):
    """wmat [128, NT*4*SLOT]: per-window packed band matrices, window-major."""
    wmat = np.zeros((NT, 128, 4 * SLOT), np.float64)
    for t in range(NT):
        rs = ROW_START[t]
        n0, n1 = assigned(t)
        for f in range(4):
            full = ops[f][n0:n1]
            mask = np.zeros(H, bool)
            mask[rs:rs + 128] = True
            assert np.abs(full[:, ~mask]).max() == 0.0, (t, f)
            wmat[t, :, f * SLOT:f * SLOT + (n1 - n0)] = full[:, rs:rs + 128].T
    return np.ascontiguousarray(wmat.transpose(1, 0, 2)).reshape(128, NT * 4 * SLOT)


# ---------------------------------------------------------------------------
# Bass kernel
# ---------------------------------------------------------------------------

class CopySched:
    """Greedy min-finish-time balancer over the three copy-capable engines."""

    def __init__(self, nc, engines=("v", "s")):
        self.nc = nc
        self.cost = {"v": 1.0417, "s": 0.8333, "g": 1.39}
        self.ovh = {"v": 170.0, "s": 210.0, "g": 300.0}
        self.load = {e: 0.0 for e in engines}

    def copy(self, dst, src, rows):
        e = min(self.load,
                key=lambda k: self.load[k] + rows * self.cost[k] + self.ovh[k])
        self.load[e] += rows * self.cost[e] + self.ovh[e]
        if e == "v":
            self.nc.vector.tensor_copy(dst, src)
        elif e == "s":
            self.nc.scalar.copy(dst, src)
        else:
            self.nc.gpsimd.tensor_copy(dst, src)


def build_nc(bpc=BPC, copy_engines=("v", "s")):
    nc = bacc.Bacc("TRN2", num_swdge_queues=4)
    x = nc.dram_tensor("x", [bpc, 128, NT * H], DT, kind="ExternalInput")
    w = nc.dram_tensor("w", [128, NT * 4 * SLOT], DT, kind="ExternalInput")
    y = nc.dram_tensor("y", [bpc, 128, 24 * 512], DT, kind="ExternalOutput")

    with tile.TileContext(nc) as tc, ExitStack() as ctx:
        const = ctx.enter_context(tc.tile_pool(name="const", bufs=1))
        xpool = ctx.enter_context(tc.tile_pool(name="xpool", bufs=2))
        t2p = ctx.enter_context(tc.tile_pool(name="t2p", bufs=2))
        outs = ctx.enter_context(tc.tile_pool(name="outs", bufs=2))
        psum = ctx.enter_context(tc.tile_pool(name="psum", bufs=1,
                                              space="PSUM"))
        sched = CopySched(nc, copy_engines)

        # w first (small; every matmul needs it), then image 0 as 9
        # per-window loads so pass-1 matmuls start after the first window
        # lands instead of after the whole 2.4MB image.
        wt = const.tile([128, NT * 4 * SLOT], DT, name="w", tag="w")
        nc.sync.dma_start(wt[:], w[:])
        x0w = []
        for t in range(NT):
            x0t = const.tile([128, H], DT, name=f"x0_{t}", tag=f"x0_{t}")
            nc.sync.dma_start(x0t[:], x[0, :, H * t:H * (t + 1)])
            x0w.append(x0t)
        xt = {}

        def emit_p1block(i, wtile):
            """Pass-1 for w-window `wtile` of image i -> t2 tile."""
            ws = ROW_START[wtile]
            ptA = psum.tile([128, 1280], F32, name="ptA", tag="ptA")
            ptB = psum.tile([128, 1024], F32, name="ptB", tag="ptB")
            for ht in range(NT):
                pt, s = (ptA, ht) if ht < 5 else (ptB, ht - 5)
                lhsT = (x0w[ht][:, ws:ws + 128] if i == 0 else
                        xt[i][:, H * ht + ws:H * ht + ws + 128])
                nc.tensor.matmul(
                    pt[:, 256 * s:256 * s + 4 * SLOT],
                    lhsT=lhsT,
                    rhs=wt[:, 4 * SLOT * ht:4 * SLOT * (ht + 1)],
                    start=True, stop=True)
            t2t = t2p.tile([128, 4 * 513], DT, name=f"t2_{wtile}",
                           tag=f"t2_{wtile}")
            t2r = t2t.rearrange("p (f s j) -> p s f j", f=4, s=NT, j=SLOT)
            srcA = ptA.rearrange("p (s c) -> p s c", c=256)[
                :, :, 0:228].rearrange("p s (f j) -> p s f j", j=SLOT)
            srcB = ptB.rearrange("p (s c) -> p s c", c=256)[
                :, :, 0:228].rearrange("p s (f j) -> p s f j", j=SLOT)
            sched.copy(t2r[:, 0:5], srcA, rows=5 * 228)
            sched.copy(t2r[:, 5:NT], srcB, rows=4 * 228)
            return t2t

        def emit_group(t2, group, rb, ot):
            ptc = {}
            for c in group:
                ptc[c] = psum.tile([128, 512], F32,
                                   name=f"pc{c}", tag="pc", bufs=3)
            for wtile in range(NT):
                n0, n1 = assigned(wtile)
                for c in group:
                    ri, ci = CHAN_OPS[c]
                    nc.tensor.matmul(
                        ptc[c][:, n0:n1],
                        lhsT=t2[wtile][:, 513 * ri + 128 * rb:
                                       513 * ri + 128 * rb + 128],
                        rhs=wt[:, 4 * SLOT * wtile + SLOT * ci:
                               4 * SLOT * wtile + SLOT * ci + (n1 - n0)],
                        start=True, stop=True)
            for c in group:
                sched.copy(ot[:, (c + 6 * rb) * 512:(c + 6 * rb) * 512 + 512],
                           ptc[c][:], rows=512)

        def make_pass2(i, t2):
            """Pass-2 of image i as a list of 20 emission closures.

            rb-major: all 6 channels of an rb row-block complete together,
            then that block's 3KB/partition slice is stored -> 4 stores
            spread across the image's pipeline, short tail.
            """
            ot = outs.tile([128, 24 * 512], DT, name="ot", tag="ot")
            work = []
            for rb in range(4):
                for group in ((0, 2), (1,), (3, 5), (4,)):
                    work.append(lambda g=group, r=rb: emit_group(t2, g, r, ot))
                lo = rb * 6 * 512
                work.append(lambda lo=lo: nc.sync.dma_start(
                    y[i, :, lo:lo + 6 * 512], ot[:, lo:lo + 6 * 512]))
            return work

        pending = []
        for i in range(bpc):
            if i + 1 < bpc:
                xt[i + 1] = xpool.tile([128, NT * H], DT, name="x", tag="x")
                nc.sync.dma_start(xt[i + 1][:], x[i + 1])
            t2 = {}
            for wtile in range(NT):
                for _ in range(3 if wtile < 2 else 2):
                    if pending:
                        pending.pop(0)()
                t2[wtile] = emit_p1block(i, wtile)
            while pending:
                pending.pop(0)()
            pending = make_pass2(i, t2)
        while pending:
            pending.pop(0)()
    return nc


_CACHED = {}


def _get_nc_and_wmat():
    if "nc" not in _CACHED:
        ops = build_ops()
        wmat = build_bands(ops).astype(NPDT)
        _CACHED["wmat"] = wmat
        nc = build_nc()
        if not nc.is_finalized():
            nc.finalize()
        _CACHED["nc"] = nc
    return _CACHED["nc"], _CACHED["wmat"]


_WIN_IDX = np.asarray(ROW_START)[:, None] + np.arange(128)[None, :]  # [NT,128]


def run(x, **spmd_kwargs):
    """x: (32, 1, 1024, 1024) float32 -> ((32, 6, 512, 512) float32, res)."""
    x = np.ascontiguousarray(np.asarray(x))
    assert x.shape == (B_TOTAL, 1, H, H), x.shape
    nc, wmat = _get_nc_and_wmat()
    in_maps = []
    for c in range(N_CORES):
        xs = x[c * BPC:(c + 1) * BPC, 0]          # [bpc, 1024, 1024]
        xw = xs[:, _WIN_IDX, :]                    # [bpc, NT, 128, 1024]
        xw = np.ascontiguousarray(
            xw.transpose(0, 2, 1, 3)).reshape(BPC, 128, NT * H)
        in_maps.append({"x": xw.astype(NPDT), "w": wmat})
    res = run_bass_kernel_spmd(nc, in_maps, list(range(N_CORES)), **spmd_kwargs)
    parts = []
    for r in res.results:
        yv = np.asarray(r["y"]).reshape(BPC, 128, 4, 6, 512)
        parts.append(yv.transpose(0, 3, 2, 1, 4).reshape(BPC, 6, 512, 512))
    return np.concatenate(parts, axis=0).astype(np.float32), res


def kernel(x):
    return run(x)[0]
